# revision 27
# baseline (speedup 1.0000x reference)
"""Causal self-attention (B=4, S=2048, D=1024, H=16) on 8 TRN2 NeuronCores.

Sharding (tensor-parallel on heads + data-parallel on batch):
  core c -> batch c//2, head-half c%2 (8 of 16 heads).
  Wq/Wk/Wv column-split, Wo row-split; the two partial outputs per batch are
  summed on the host (+ bo), which is the row-parallel unshard.

Per-core Bass/Tile program (matmul operands bf16, psum/softmax fp32), built
around keeping the PE stream dense (HAM stays at K=8/8) and the ScalarE exp
stream saturated:

  prologue: q/k projections for head-pair 0 and v for token tiles 0..7.
  main loop (hp outer, superblock i inner, key tile j innermost):
    scores for both heads of the pair land in one 2-bank psum tile
    ([128, 1024], row-group tile_position packing); ONE exp activation per
    key tile covers both heads; diagonal-tile causal masking is a single
    GpSimd affine_select that zeroes the upper triangle of probs (garbage
    from the skipped dead columns is zeroed by the same select); PV (ones
    column producing sumexp in row 64) runs one key tile behind the exp.
    PE idle slots during the ScalarE-paced stretches are filled with v
    projections (hp 0), the next head-pair's q/k projections (hp 0..2) and
    the output projection for completed superblocks (hp 3).
  normalize (deferred one (hp, i) unit): reciprocal_approx_fast on the
    sumexp rows, GpSimd partition_broadcast, DVE multiply into attnT.
  phase C: out_partial = attnT.T @ Wo_rows per 128-token tile.
"""

from collections import deque
from contextlib import ExitStack

import numpy as np
import ml_dtypes

import concourse.bass as bass
import concourse.bacc as bacc
import concourse.tile as tile
import concourse.mybir as mybir

F32 = mybir.dt.float32
F32R = mybir.dt.float32r
BF16 = mybir.dt.bfloat16


def build_core_program(S=2048, D=1024, HC=8, DH=64, SQ=512):
    """Build the per-core Bass program (SPMD: same program, different data).
    The host must pass xT/wqk/wv/wo as bfloat16 arrays."""
    DQ = HC * DH              # head-slice width (512)
    DK = D // 128             # contraction tiles for projections (8)
    DQN = DQ // 128           # head-pair count (4)
    NSB = S // SQ             # query superblocks (4)
    NTT = S // 128            # token tiles (16)
    ND = SQ // 128            # key tiles per superblock (4)
    assert DQ % 128 == 0 and S % SQ == 0 and SQ % 128 == 0 and D % 128 == 0

    nc = bacc.Bacc("TRN2", target_bir_lowering=False, debug=False)

    xT = nc.dram_tensor("xT", [D, S], BF16, kind="ExternalInput").ap()
    wqk = nc.dram_tensor("wqk", [D, 2 * DQ], BF16, kind="ExternalInput").ap()
    wv = nc.dram_tensor("wv", [D, DQ], BF16, kind="ExternalInput").ap()
    wo = nc.dram_tensor("wo", [DQ, D], BF16, kind="ExternalInput").ap()
    bqk = nc.dram_tensor("bqk", [2 * DQ], F32, kind="ExternalInput").ap()
    bv = nc.dram_tensor("bv", [DQ], F32, kind="ExternalInput").ap()
    out = nc.dram_tensor("out", [S, D], BF16, kind="ExternalOutput").ap()

    with tile.TileContext(nc) as tc, ExitStack() as ctx:
        ctx.enter_context(nc.allow_low_precision(
            reason="low-precision matmul operands; accumulation stays fp32"))
        const = ctx.enter_context(tc.tile_pool(name="const", bufs=1))
        big = ctx.enter_context(tc.tile_pool(name="big", bufs=1))
        stream = ctx.enter_context(tc.tile_pool(name="stream", bufs=1))
        psum = ctx.enter_context(tc.tile_pool(name="psum", bufs=1, space="PSUM"))

        # ---- constants ----
        ones_hc = const.tile([128, HC], F32)
        nc.vector.memset(ones_hc[:], 1.0)
        # binary causal mask for the 128-wide diagonal boundary subtile:
        # 1 where query >= key else 0 (multiplied into probs on DVE)
        tri01 = const.tile([128, 128], BF16)
        nc.vector.memset(tri01[:], 1.0)
        nc.gpsimd.affine_select(
            out=tri01[:], in_=tri01[:], compare_op=mybir.AluOpType.is_ge,
            fill=0.0, base=0, channel_multiplier=-1, pattern=[[1, 128]])

        # biases: bqk as [128, 2*DQN] (column t = dout tile t), bv broadcast
        bqk_sb = const.tile([128, 2 * DQN], F32)
        nc.gpsimd.dma_start(bqk_sb[:], bqk.rearrange("(t p) -> p t", p=128))
        bv_rowf = const.tile([1, DQ], F32)
        nc.gpsimd.dma_start(bv_rowf[:], bv.rearrange("(a d) -> a d", a=1))
        bv_bc = const.tile([128, DQ], F32)
        nc.gpsimd.partition_broadcast(bv_bc[:], bv_rowf[:])

        # ---- big resident tensors ----
        xt_all = big.tile([128, DK, S], BF16)
        wqk_sb = big.tile([128, DK, 2 * DQ], BF16)
        wv_sb = big.tile([128, DK, DQ], BF16)
        wo_sb = big.tile([128, DQN, D], BF16)
        kT = big.tile([128, DQN, S], BF16)      # [pair 2x64 rows, tokens]
        qT = big.tile([128, DQN, S], BF16)
        v_aug = big.tile([128, NTT, HC * 65], BF16)
        attnT = big.tile([128, DQN, S], BF16)

        # first-needed-first load order, striped across the three DMA
        # dispatch queues: the prologue is device-HBM-bound (all 8 cores
        # load at once), so the kt-ascending xt/wqk pieces the first
        # projection chains consume must not sit behind bulk loads
        for kt in range(DK):
            r = slice(128 * kt, 128 * (kt + 1))
            nc.sync.dma_start(xt_all[:, kt, 0:S // 2], xT[r, 0:S // 2])
            nc.scalar.dma_start(wqk_sb[:, kt, :], wqk[r, :])
            nc.gpsimd.dma_start(wv_sb[:, kt, :], wv[r, :])
        for kt in range(DK):
            r = slice(128 * kt, 128 * (kt + 1))
            (nc.sync if kt % 2 == 0 else nc.scalar).dma_start(
                xt_all[:, kt, S // 2:S], xT[r, S // 2:S])
        for p4 in range(DQN):
            nc.gpsimd.dma_start(wo_sb[:, p4, :],
                                wo[128 * p4:128 * (p4 + 1), :])

        # ---- work units (emitted inline or as 2-chunk fillers) -----------
        # fillers are split into ~4-8 matmul chunks: the Tile scheduler
        # drops a whole ready filler into any PE-free moment at unit
        # boundaries, and oversized fillers overshoot the gap and stall
        # the scores -> exp stream behind them
        def proj_unit(dt, tbs):
            # q/k projection: out-dim block dt, token superblocks in tbs
            state = {}

            def chunk(ck):
                def emit():
                    if ck == 0:
                        state['pss'] = {
                            tb: psum.tile([128, SQ], F32, tag="misc", bufs=2,
                                          name=f"pp_{dt}_{tb}")
                            for tb in tbs}
                    for kt in range(2 * ck, 2 * ck + 2):
                        for tb in tbs:
                            nc.tensor.matmul(
                                state['pss'][tb][:],
                                wqk_sb[:, kt, 128 * dt:128 * (dt + 1)],
                                xt_all[:, kt, tb * SQ:(tb + 1) * SQ],
                                start=(kt == 0), stop=(kt == DK - 1))
                    if ck == DK // 2 - 1:
                        is_q = dt < DQN
                        hp = dt % DQN
                        dest = qT if is_q else kT
                        for tb in tbs:
                            nc.vector.tensor_scalar(
                                dest[:, hp, tb * SQ:(tb + 1) * SQ],
                                state['pss'][tb][:],
                                0.125 if is_q else 1.0, bqk_sb[:, dt:dt + 1],
                                op0=mybir.AluOpType.mult,
                                op1=mybir.AluOpType.add)
                return emit
            return [chunk(ck) for ck in range(DK // 2)]

        def v_unit(tt):
            # v projection for one 128-token tile (token-stationary)
            state = {}

            def chunk_a():
                state['psv'] = psum.tile([128, DQ], F32, tag="misc", bufs=2,
                                         name=f"pv_{tt}")
                for kt in range(DK // 2):
                    nc.tensor.matmul(
                        state['psv'][:],
                        xt_all[:, kt, 128 * tt:128 * (tt + 1)],
                        wv_sb[:, kt, :], start=(kt == 0), stop=False)

            def chunk_b():
                psv = state['psv']
                for kt in range(DK // 2, DK):
                    nc.tensor.matmul(
                        psv[:], xt_all[:, kt, 128 * tt:128 * (tt + 1)],
                        wv_sb[:, kt, :], start=False, stop=(kt == DK - 1))
                va = v_aug[:, tt, :].rearrange("p (h c) -> p h c", h=HC)
                nc.vector.tensor_tensor(
                    va[:, :, 0:64], psv[:].rearrange("p (h c) -> p h c", h=HC),
                    bv_bc[:].rearrange("p (h c) -> p h c", h=HC),
                    op=mybir.AluOpType.add)
                nc.vector.tensor_copy(va[:, :, 64:65], ones_hc[:, :, None])
            return [chunk_a, chunk_b]

        def phase_c_unit(tt, tag="misc"):
            # output projection for one 128-token tile, one nb per chunk
            def chunk(nb):
                def emit():
                    pos = psum.tile([128, SQ], F32, tag=tag, bufs=2,
                                    name=f"po_{tt}_{nb}")
                    for p4 in range(DQN):
                        nc.tensor.matmul(
                            pos[:],
                            attnT[:, p4, 128 * tt:128 * (tt + 1)],
                            wo_sb[:, p4, nb * SQ:(nb + 1) * SQ],
                            start=(p4 == 0), stop=(p4 == DQN - 1))
                    osb = stream.tile([128, SQ], BF16, tag="osb", bufs=3,
                                      name=f"ob_{tt}_{nb}")
                    nc.vector.tensor_copy(osb[:], pos[:])
                    nc.sync.dma_start(
                        out[128 * tt:128 * (tt + 1),
                            nb * SQ:(nb + 1) * SQ], osb[:])
                return emit
            return [chunk(0), chunk(1)]

        def make_norm(hp, i, pva, pvb):
            # deferred: 1/sumexp, partition-broadcast, scale into attnT
            def emit():
                for hh, pv in ((0, pva), (1, pvb)):
                    # custom-DVE ops mishandle non-zero partition offsets:
                    # evacuate the sumexp row to a partition-0 SBUF tile
                    # with a standard copy before reciprocal_approx_fast
                    se = stream.tile([1, SQ], F32, tag="se", bufs=4,
                                     name=f"se_{hp}_{i}_{hh}")
                    nc.vector.tensor_copy(se[:], pv[64:65, :])
                    rc = stream.tile([1, SQ], F32, tag="recip", bufs=4,
                                     name=f"rc_{hp}_{i}_{hh}")
                    nc.vector.reciprocal_approx_fast(rc[:], se[:])
                    bc = stream.tile([64, SQ], F32, tag="bc", bufs=4,
                                     name=f"bn_{hp}_{i}_{hh}")
                    nc.gpsimd.partition_broadcast(bc[:], rc[:])
                    if hh == 0:
                        nc.vector.tensor_tensor(
                            attnT[0:64, hp, i * SQ:(i + 1) * SQ],
                            pv[0:64, :], bc[:], op=mybir.AluOpType.mult)
                    else:
                        stage = stream.tile([64, SQ], BF16, tag="stage",
                                            bufs=2, name=f"st_{hp}_{i}")
                        nc.vector.tensor_tensor(
                            stage[:], pv[0:64, :], bc[:],
                            op=mybir.AluOpType.mult)
                        nc.sync.dma_start(
                            attnT[64:128, hp, i * SQ:(i + 1) * SQ], stage[:])
            return emit

        # ---- prologue: head-pair 0 projections + first half of v ---------
        for u in [proj_unit(0, [0]), proj_unit(DQN, [0]),  # q/k hp0, sb 0
                  v_unit(0), v_unit(1)]:
            for c in u:
                c()

        # ---- main loop ----------------------------------------------------
        fillers = deque()
        for u in [v_unit(2), v_unit(3),
                  proj_unit(0, [1]), proj_unit(DQN, [1]),  # q/k hp0, sb 1
                  v_unit(4), v_unit(5), v_unit(6), v_unit(7),
                  proj_unit(0, [2, 3]), proj_unit(DQN, [2, 3]),
                  v_unit(8), v_unit(9), v_unit(10), v_unit(11),
                  proj_unit(1, [0, 1]), proj_unit(DQN + 1, [0, 1]),
                  v_unit(12), v_unit(13), v_unit(14), v_unit(15),
                  proj_unit(1, [2, 3]), proj_unit(DQN + 1, [2, 3])]:
            fillers.extend(u)
        pend_norm = [None]

        for hp in range(DQN):
            if 1 <= hp < DQN - 1:
                ndt = hp + 1
                for pair in range(2):
                    tbs = [2 * pair, 2 * pair + 1]
                    fillers.extend(proj_unit(ndt, tbs))
                    fillers.extend(proj_unit(DQN + ndt, tbs))
            for i in range(NSB):
                if hp == DQN - 1 and i >= 1:
                    # all heads' attnT for superblock i-1 is complete
                    for m in range(ND):
                        fillers.extend(phase_c_unit((i - 1) * ND + m))
                NJ = ND * (i + 1)
                pva = psum.tile([65, SQ], F32, tag="pv", bufs=2,
                                name=f"pa_{hp}_{i}")
                pvb = psum.tile([65, SQ], F32, tag="pv", bufs=2,
                                name=f"pb_{hp}_{i}")
                pend_pv = None
                for j in range(NJ):
                    jj = j - ND * i
                    f0 = max(0, 128 * jj)
                    sc = psum.tile([128, 2 * SQ], F32, tag="sc", bufs=2,
                                   name=f"sc_{hp}_{i}_{j}")
                    probs = stream.tile([128, 2 * SQ], BF16, tag="probs",
                                        bufs=8, name=f"pr_{hp}_{i}_{j}")
                    for hh in range(2):
                        p0 = 64 * hh
                        g0 = f0 if hh == 0 else 0
                        nc.tensor.matmul(
                            sc[:, hh * SQ + g0:(hh + 1) * SQ],
                            kT[p0:p0 + 64, hp, 128 * j:128 * (j + 1)],
                            qT[p0:p0 + 64, hp, i * SQ + g0:(i + 1) * SQ],
                            start=True, stop=True,
                            tile_position=(p0, 0))
                    nc.scalar.activation(
                        probs[:, f0:], sc[:, f0:],
                        mybir.ActivationFunctionType.Exp)
                    if jj >= 0:
                        # zero probs where query < key; only the 128-wide
                        # boundary subtile matters — PV reads probs[:, f0:],
                        # so columns left of f0 are never consumed
                        for hh in range(2):
                            pr = probs[:, hh * SQ + f0:hh * SQ + f0 + 128]
                            nc.gpsimd.affine_select(
                                out=pr, in_=pr,
                                compare_op=mybir.AluOpType.is_ge,
                                fill=0.0, base=0, channel_multiplier=-1,
                                pattern=[[1, 128]])
                    if j == 1 and pend_norm[0] is not None:
                        pend_norm[0]()
                        pend_norm[0] = None
                    if pend_pv is not None:
                        pj, pf0, pprobs = pend_pv
                        for hh, pv in ((0, pva), (1, pvb)):
                            h = 2 * hp + hh
                            nc.tensor.matmul(
                                pv[:, pf0:],
                                v_aug[:, pj, 65 * h:65 * h + 65],
                                pprobs[:, hh * SQ + pf0:(hh + 1) * SQ],
                                start=(pj == 0), stop=(pj == NJ - 1))
                    pend_pv = (j, f0, probs)
                    npop = 0
                    if hp == 0:
                        npop = (3 if i == 0 else 2) if i <= 1 else 1
                    elif hp == DQN - 1:
                        npop = 1 if j >= 2 else 0
                    else:
                        npop = 2 if j % 2 == 1 else 0
                    while fillers and npop > 0:
                        fillers.popleft()()
                        npop -= 1
                pj, pf0, pprobs = pend_pv
                for hh, pv in ((0, pva), (1, pvb)):
                    h = 2 * hp + hh
                    nc.tensor.matmul(
                        pv[:, pf0:],
                        v_aug[:, pj, 65 * h:65 * h + 65],
                        pprobs[:, hh * SQ + pf0:(hh + 1) * SQ],
                        start=(pj == 0), stop=(pj == NJ - 1))
                pend_norm[0] = make_norm(hp, i, pva, pvb)

        # ---- tail: last normalize, remaining fillers, last superblock ----
        if pend_norm[0] is not None:
            pend_norm[0]()
            pend_norm[0] = None
        while fillers:
            fillers.popleft()()
        for m in range(ND):
            for c in phase_c_unit((NSB - 1) * ND + m,
                                  tag="sc" if m % 2 else "misc"):
                c()

    nc.compile()
    return nc


B, S, D, H = 4, 2048, 1024, 16
N_CORES = 8

_CACHED = {}


def _make_core_inputs(x, Wq, bq, Wk, bk, Wv, bv, Wo):
    DQ = D // 2

    def cast(a):
        return np.ascontiguousarray(a).astype(ml_dtypes.bfloat16)

    xTs = [cast(x[b].T) for b in range(B)]
    in_maps = []
    for c in range(N_CORES):
        b, hf = c // 2, c % 2
        sl = slice(hf * DQ, (hf + 1) * DQ)
        in_maps.append({
            "xT": xTs[b],
            "wqk": cast(np.concatenate([Wq[:, sl], Wk[:, sl]], axis=1)),
            "wv": cast(Wv[:, sl]),
            "wo": cast(Wo[sl, :]),
            "bqk": np.ascontiguousarray(
                np.concatenate([0.125 * bq[sl], bk[sl]])).astype(np.float32),
            "bv": np.ascontiguousarray(bv[sl]).astype(np.float32),
        })
    return in_maps


def kernel(x, Wq, bq, Wk, bk, Wv, bv, Wo, bo):
    import tempfile
    from concourse import bass_utils

    x = np.asarray(x, dtype=np.float32)
    Wq = np.asarray(Wq, dtype=np.float32)
    bq = np.asarray(bq, dtype=np.float32)
    Wk = np.asarray(Wk, dtype=np.float32)
    bk = np.asarray(bk, dtype=np.float32)
    Wv = np.asarray(Wv, dtype=np.float32)
    bv = np.asarray(bv, dtype=np.float32)
    Wo = np.asarray(Wo, dtype=np.float32)
    bo = np.asarray(bo, dtype=np.float32)

    if "nc" not in _CACHED:
        _CACHED["nc"] = build_core_program(S=S, D=D, HC=H // 2)
    nc = _CACHED["nc"]

    in_maps = _make_core_inputs(x, Wq, bq, Wk, bk, Wv, bv, Wo)
    res = bass_utils.run_bass_kernel_spmd(
        nc, in_maps, core_ids=list(range(N_CORES)),
        tmpdir=tempfile.mkdtemp(prefix="bass_attn_"))

    out = np.empty((B, S, D), dtype=np.float32)
    for b in range(B):
        out[b] = (res.results[2 * b]["out"].astype(np.float32)
                  + res.results[2 * b + 1]["out"].astype(np.float32) + bo)
    return out
